# revision 2
# baseline (speedup 1.0000x reference)
"""MultiEmbedding (embedding_lookup) Trainium2 kernel.

Math: y[n, :] = sum_l weight[l, x[n, l], :]  for x1/x2 [65536, 8], weight [8, 1024, 1024].

Strategy (8 NeuronCores, data-parallel over tokens):
  - Concat x1+x2 -> 131072 tokens, 16384 per core.
  - Host packs flat indices (l*1024 + x) into the int16 wrapped layout that
    nc.gpsimd.dma_gather expects ([128, S], idx i at partition i%16 slot i//16,
    replicated across the 8 partition groups).
  - Device: dma_gather pulls 4KB rows (1024 f32) from the flattened [8192, 1024]
    HBM table into SBUF tiles laid out [128 tok-part, levels, 1024]; DVE
    accumulates the 8 level rows per token sequentially (bit-exact vs the
    reference sum order); results DMA back to HBM.
"""

import numpy as np

L, K, D = 8, 1024, 1024
T_TOTAL = 131072  # x1 + x2 tokens
N_CORES = 8
T_CORE = T_TOTAL // N_CORES  # 16384

# tunables (HW-swept: 128tok/4buf/1queue -> ~1.8ms; 256/2/1 -> 2.33ms;
# 2 queues regressed to 3.0ms)
CHUNK_TOK = 128  # tokens per dma_gather
GBUFS = 4  # gather-tile buffering
N_QUEUES = 1  # SWDGE queues used round-robin

IDX_SLOTS = T_CORE * L // 16  # 8192 free-dim slots of the idx tile

_compiled = None


def _build(chunk_tok=CHUNK_TOK, gbufs=GBUFS, n_queues=N_QUEUES):
    import concourse.bass as bass
    import concourse.tile as tile
    from concourse import bacc, mybir

    n_chunks = T_CORE // chunk_tok
    num_idxs = chunk_tok * L
    blocks = chunk_tok // 128  # 128-token blocks per chunk

    nc = bacc.Bacc(
        "TRN2",
        target_bir_lowering=False,
        debug=False,
        num_devices=N_CORES,
        num_swdge_queues=n_queues,
    )
    w_ap = nc.dram_tensor("w", [L * K, D], mybir.dt.float32, kind="ExternalInput").ap()
    idx_ap = nc.dram_tensor(
        "idx", [128, IDX_SLOTS], mybir.dt.int16, kind="ExternalInput"
    ).ap()
    y_ap = nc.dram_tensor(
        "y", [T_CORE, D], mybir.dt.float32, kind="ExternalOutput"
    ).ap()

    with tile.TileContext(nc) as tc:
        with (
            tc.tile_pool(name="const", bufs=1) as const_pool,
            tc.tile_pool(name="g", bufs=gbufs) as gpool,
            tc.tile_pool(name="yt", bufs=gbufs) as ypool,
        ):
            idx_sb = const_pool.tile([128, IDX_SLOTS], mybir.dt.int16)
            nc.sync.dma_start(out=idx_sb[:], in_=idx_ap)

            slots_per_chunk = num_idxs // 16  # idx free-dim slots per chunk
            for k in range(n_chunks):
                g = gpool.tile([128, blocks * L, D], mybir.dt.float32)
                nc.gpsimd.dma_gather(
                    out_ap=g[:],
                    in_ap=w_ap,
                    idxs_ap=idx_sb[
                        :, k * slots_per_chunk : (k + 1) * slots_per_chunk
                    ],
                    num_idxs=num_idxs,
                    num_idxs_reg=num_idxs,
                    elem_size=D,
                    single_packet=False,
                    queue_num=k % n_queues,
                )
                yt = ypool.tile([128, blocks, D], mybir.dt.float32)
                for b in range(blocks):
                    # sequential l=0..7 accumulation (bit-exact vs reference
                    # sum order), unit-stride adds on DVE
                    s = b * L
                    nc.vector.tensor_add(
                        out=yt[:, b, :], in0=g[:, s, :], in1=g[:, s + 1, :]
                    )
                    for l in range(2, L):
                        nc.vector.tensor_add(
                            out=yt[:, b, :], in0=yt[:, b, :], in1=g[:, s + l, :]
                        )
                nc.sync.dma_start(
                    out=y_ap[k * chunk_tok : (k + 1) * chunk_tok].rearrange(
                        "(b p) d -> p b d", p=128
                    ),
                    in_=yt[:],
                )
    nc.compile()
    return nc


def _get_compiled():
    global _compiled
    if _compiled is None:
        _compiled = _build()
    return _compiled


def _pack_indices(x_core: np.ndarray) -> np.ndarray:
    """x_core [T_CORE, 8] int -> [128, IDX_SLOTS] int16 dma_gather layout."""
    flat = (x_core.astype(np.int64) + (np.arange(L, dtype=np.int64) * K)[None, :]).astype(
        np.int16
    )
    # ordering i = (t//128)*1024 + l*128 + (t%128)
    q = flat.reshape(T_CORE // 128, 128, L).transpose(0, 2, 1).reshape(-1)
    # wrapped: idx_tile[p, s] = q[s*16 + p%16], replicated over 8 groups of 16
    qr = q.reshape(-1, 16).T  # [16, IDX_SLOTS]
    return np.tile(qr, (8, 1)).copy()  # [128, IDX_SLOTS]


def _prepare_in_maps(x1: np.ndarray, x2: np.ndarray, weight: np.ndarray):
    x = np.concatenate([np.asarray(x1), np.asarray(x2)], axis=0)
    w_flat = np.ascontiguousarray(np.asarray(weight), dtype=np.float32).reshape(
        L * K, D
    )

    in_maps = []
    for c in range(N_CORES):
        xc = x[c * T_CORE : (c + 1) * T_CORE]
        in_maps.append({"w": w_flat, "idx": _pack_indices(xc)})
    return in_maps


def kernel(x1: np.ndarray, x2: np.ndarray, weight: np.ndarray):
    from concourse.bass_utils import run_bass_kernel_spmd

    nc = _get_compiled()
    in_maps = _prepare_in_maps(x1, x2, weight)

    res = _run_with_retry(run_bass_kernel_spmd, nc, in_maps)
    y_full = np.concatenate([res.results[c]["y"] for c in range(N_CORES)], axis=0)
    return (y_full[: T_TOTAL // 2], y_full[T_TOTAL // 2 :])


def _retry_call(fn, attempts=6, sleep_s=75):
    """The axon-tunnelled device occasionally reports unrecoverable for a few
    minutes after a previous session crashed; back off and retry."""
    import time

    last = None
    for i in range(attempts):
        try:
            return fn()
        except Exception as e:  # noqa: BLE001 - jax.errors.JaxRuntimeError etc.
            last = e
            if i == attempts - 1:
                break
            try:
                import jax

                jax.clear_caches()
                import jax.extend.backend

                jax.extend.backend.clear_backends()
            except Exception:
                pass
            time.sleep(sleep_s)
    raise last


def _run_with_retry(run_fn, nc, in_maps, attempts=6, sleep_s=75):
    return _retry_call(
        lambda: run_fn(nc, in_maps, core_ids=list(range(N_CORES))),
        attempts=attempts,
        sleep_s=sleep_s,
    )

